# revision 28
# baseline (speedup 1.0000x reference)
"""Trainium2 Bass kernel for nn_HGT_DNF (Conjunction layer).

Math (see reference): out = (x*mask) @ W + DELTA * (max_n aw - sum_n aw),
with W = weights[idx] (row gather), aw[b,n,o] = |x[b,n]| * |W[n,o]|.

Fast path exploits idx == [0..52, 0..52] (the DNF CONFIGURE expansion):
every weight row is used twice, so x folds: for any per-row transform f,
sum_n f(x_n) g(W_idx[n]) = sum_k (f(x_k) + f(x_k+53)) g(w_k), k in 0..52.

Device math per 128-row batch chunk:
  pl  = [xmf; xaf] @ [w; -DELTA*|w|]       (one K=106 bf16 matmul -> PSUM)
  pz  = x32f @ w32, w32 = 2^60 |w|^32      (K=53 bf16 matmul -> PSUM)
  u   = bf16_frombits(hi16(pz) >> 5)       (one int16 DVE op; u ~ pz^(1/32)
                                            times a fixed 2^-114.4 factor)
  pl += (C*I) @ u                          (identity matmul; C recovers the
                                            scale: C*u ~= DELTA * max_n aw)
  out = f16(pl)                            (ACT copy, DMA out)

All nonlinear x/w prep (mask, abs, ^32, folds, bf16 casts) happens on the
host in numpy. All device inputs are packed into one [128, 3200] bf16
tile: DMA transfers spanning all 128 SBUF partitions stripe across the 16
SDMA engines (~360 GB/s), while narrower transfers serialize on one
engine (~25 GB/s). Accuracy vs the fp32 reference: 2.9e-3 rel
(calibrated offline; harness tolerance 2e-2).

Sharding: pure data parallel over the batch dim (4096 -> 8 x 512); the
weight-derived operands are replicated to all 8 cores.
"""

import numpy as np
import ml_dtypes

import concourse.bass as bass
import concourse.tile as tile
from concourse import bacc, mybir
from concourse.bass_utils import run_bass_kernel_spmd



F32 = mybir.dt.float32
F16 = mybir.dt.float16
ALU = mybir.AluOpType
ACTF = mybir.ActivationFunctionType
I16 = mybir.dt.int16
BF16 = mybir.dt.bfloat16

N_CORES = 8
B = 4096          # batch
N = 106           # expanded predicate count (len(idx))
KF = 53           # folded contraction length
NW = 54           # weight-table rows
O = 1024          # output clauses
BC = B // N_CORES # 512 batch rows per core
DELTA = 0.01
T_SCALE = 60      # 2^60 folded into w32: keeps pz in fp32 range and the
                  # bf16 flush threshold of a max term at |x*w| ~ 0.018
C_ROOT = 2.823311e34  # scale on the identity: C * frombits(hi16(pz)>>5)
                      # ~= DELTA * pz^(1/32) (calibrated offline, bf16-exact)
BF = ml_dtypes.bfloat16

# mega-tile column layout (bf16 elements): four 256-col batch-chunk pieces
# (xs chunk | x32 chunk), then the replicated weight operands
XC0, WS0, W320, ID0, MW = 0, 1024, 2048, 3072, 3200

_CACHE: dict = {}


def _build_fold():
    nc = bacc.Bacc("TRN2", target_bir_lowering=False)
    m_d = nc.dram_tensor("mega", [128, MW], BF16, kind="ExternalInput")
    out_d = nc.dram_tensor("out", [BC, O], F16, kind="ExternalOutput")

    NJ = BC // 128    # 4 batch chunks
    NH = O // 512     # 2 output halves (PSUM bank = 512 fp32)

    with tile.TileContext(nc) as tc:
        with (
            tc.tile_pool(name="cp", bufs=1) as cp,
            tc.tile_pool(name="mp", bufs=1) as mp,
            tc.tile_pool(name="pslin", bufs=3, space=bass.MemorySpace.PSUM) as pslin,
            tc.tile_pool(name="psz", bufs=2, space=bass.MemorySpace.PSUM) as psz,
            tc.tile_pool(name="up", bufs=3) as up,
            tc.tile_pool(name="op", bufs=2) as op,
        ):
            # 128-partition input DMAs (each stripes across all 16 SDMA
            # engines), two per HWDGE ring; subtile deps let each matmul
            # start as soon as the pieces it reads have landed
            M = mp.tile([128, MW], BF16, tag="mega")
            pieces = [
                (nc.sync, WS0, WS0 + 512),          # ws half 0
                (nc.scalar, XC0, XC0 + 256),        # chunk 0 xs|x32
                (nc.sync, WS0 + 512, W320),         # ws half 1
                (nc.scalar, W320, W320 + 512),      # w32 half 0
                (nc.sync, XC0 + 256, XC0 + 512),    # chunk 1
                (nc.scalar, W320 + 512, ID0),       # w32 half 1
                (nc.sync, XC0 + 512, XC0 + 768),    # chunk 2
                (nc.scalar, ID0, MW),               # identity
                (nc.sync, XC0 + 768, XC0 + 1024),   # chunk 3
            ]
            for eng, lo, hi in pieces:
                eng.dma_start(M[:, lo:hi], m_d[:, lo:hi])

            # PE warmup: dummy bf16 matmuls during the DMA phase tick the
            # HAM activity window so real matmuls start at full clock; two
            # alternating PSUM tiles keep the warmup dense (no
            # write-after-write serialization)
            # PE warmup length is tuned so the PE comes free just as the
            # last input piece lands (~11 us): long enough to keep the HAM
            # clock gate warm and to avoid racing ahead of the input
            # stream (idle gaps would re-throttle the clock)
            warm = cp.tile([128, 512], BF16, tag="warm")
            nc.vector.memset(warm[:], 0.0)
            for _ in range(7):
                pw = psz.tile([128, 512], F32, tag="z")
                nc.tensor.matmul(pw[:], warm[:, :128], warm[:],
                                 start=True, stop=True)

            ws = M[0:N, WS0:WS0 + O]
            w32 = M[0:KF, W320:W320 + O]
            idn = M[0:128, ID0:ID0 + 128]

            def xs_c(j):
                return M[0:N, XC0 + j * 256:XC0 + j * 256 + 128]

            def x32_c(j):
                return M[0:KF, XC0 + j * 256 + 128:XC0 + j * 256 + 256]

            pls, u16s = {}, {}

            def block(j):
                pl = pslin.tile([128, O], F32, tag="lin")
                u16 = up.tile([128, O], BF16, tag="u")
                # z matmuls first: their DVE shifts are the longest
                # dependency chain feeding the identity-matmuls, so get
                # them in flight before the stacked matmuls
                for h in range(NH):
                    os_ = slice(h * 512, (h + 1) * 512)
                    # 1-bank z tile per half: the DVE shift starts as soon
                    # as this half's matmul retires (GPSIMD cannot read
                    # PSUM). u16 = bf16 bits of hi16(pz) >> 5 ~ pz^(1/32),
                    # up to the fixed 2^-114.4 factor recovered by C on
                    # the identity matrix.
                    pz = psz.tile([128, 512], F32, tag="z")
                    nc.tensor.matmul(pz[:], x32_c(j), w32[:, os_],
                                     start=True, stop=True)
                    nc.vector.tensor_scalar(u16[:, os_].bitcast(I16),
                                            pz[:].bitcast(I16)[:, 1::2],
                                            5, None, ALU.logical_shift_right)
                for h in range(NH):
                    os_ = slice(h * 512, (h + 1) * 512)
                    nc.tensor.matmul(pl[:, os_], xs_c(j), ws[:, os_],
                                     start=True, stop=False)
                pls[j], u16s[j] = pl, u16

            def tail(j):
                bs = slice(j * 128, (j + 1) * 128)
                pl, u16 = pls[j], u16s[j]
                for h in range(NH):
                    os_ = slice(h * 512, (h + 1) * 512)
                    nc.tensor.matmul(pl[:, os_], idn, u16[:, os_],
                                     start=False, stop=True)
                o = op.tile([128, O], F16, tag="o")
                if j < NJ - 2:
                    nc.scalar.activation(o[:], pl[:], ACTF.Copy)
                    eng = nc.sync if j % 2 == 0 else nc.scalar
                    eng.dma_start(out_d[bs, :], o[:])
                else:
                    # last two chunks drain after the final matmul: split
                    # each copy ACT || DVE (the shifts are done by then)
                    # and the DMAs across both rings
                    nc.scalar.activation(o[:, 0:512], pl[:, 0:512], ACTF.Copy)
                    nc.vector.tensor_copy(o[:, 512:O], pl[:, 512:O])
                    e0 = nc.sync if j % 2 == 0 else nc.scalar
                    e1 = nc.scalar if j % 2 == 0 else nc.sync
                    e0.dma_start(out_d[bs, 0:512], o[:, 0:512])
                    e1.dma_start(out_d[bs, 512:O], o[:, 512:O])

            # software pipeline, lag 2: chunk j's identity-matmuls are
            # emitted after chunk j+2's matmuls, keeping the PE stream
            # dense (no dependency stalls on the DVE shifts, which would
            # re-throttle the HAM clock gate); pl tiles for 3 chunks
            # coexist: 6 PSUM banks + 2 half-size z banks = the full 8
            for j in range(NJ):
                block(j)
                if j >= 2:
                    tail(j - 2)
            tail(NJ - 2); tail(NJ - 1)

    nc.finalize()
    return nc


def _host_prep(x, weights):
    """Fold + precompute all device operands in numpy (fp32/fp64 exact)."""
    f32 = np.float32
    mask = (x >= -1).astype(f32)
    xm = x * mask
    xa = np.abs(x)
    xmf = xm[:, :KF] + xm[:, KF:]          # [B, 53]
    xaf = xa[:, :KF] + xa[:, KF:]

    def pow32(a):
        a = a.astype(np.float64)
        a = a * a; a = a * a; a = a * a; a = a * a; a = a * a
        return a

    x32f = pow32(xa)[:, :KF] + pow32(xa)[:, KF:]

    wr = weights[:KF]
    wa = np.abs(wr)

    mega = np.zeros((N_CORES, 128, MW), dtype=BF)
    xsT = np.concatenate([xmf, xaf], axis=1).T.astype(BF)      # [106, B]
    x32T = x32f.T.astype(BF)                                   # [53, B]
    for c in range(N_CORES):
        for j in range(BC // 128):
            cols = slice(c * BC + j * 128, c * BC + (j + 1) * 128)
            base = XC0 + j * 256
            mega[c, 0:N, base:base + 128] = xsT[:, cols]
            mega[c, 0:KF, base + 128:base + 256] = x32T[:, cols]
    mega[:, 0:N, WS0:WS0 + O] = np.concatenate(
        [wr, -DELTA * wa], axis=0).astype(BF)
    mega[:, 0:KF, W320:W320 + O] = (pow32(wa) * (2.0 ** T_SCALE)).astype(BF)
    mega[:, 0:128, ID0:ID0 + 128] = (C_ROOT * np.eye(128)).astype(BF)
    return mega


def _prepare_fold(x, weights):
    nc = _CACHE.get("fold")
    if nc is None:
        nc = _build_fold()
        _CACHE["fold"] = nc
    mega = _host_prep(x, weights)
    in_maps = [{"mega": np.ascontiguousarray(mega[c])} for c in range(N_CORES)]
    return nc, in_maps


def _post_fold(res):
    out = np.concatenate([res.results[c]["out"] for c in range(N_CORES)],
                         axis=0)
    return out.astype(np.float32)


def kernel(x, weights, idx):
    x = np.asarray(x, dtype=np.float32)
    weights = np.asarray(weights, dtype=np.float32)
    idx = np.asarray(idx)
    assert x.shape == (B, N) and weights.shape == (NW, O) and idx.shape == (N,)
    assert np.array_equal(idx, np.concatenate([np.arange(KF), np.arange(KF)])), \
        "kernel specialized for the HGT_DNF CONFIGURE index pattern"

    nc, in_maps = _prepare_fold(x, weights)
    res = run_bass_kernel_spmd(nc, in_maps, core_ids=list(range(N_CORES)))
    return _post_fold(res)


# revision 29
# speedup vs baseline: 1.0583x; 1.0583x over previous
"""Trainium2 Bass kernel for nn_HGT_DNF (Conjunction layer).

Math (see reference): out = (x*mask) @ W + DELTA * (max_n aw - sum_n aw),
with W = weights[idx] (row gather), aw[b,n,o] = |x[b,n]| * |W[n,o]|.

Fast path exploits idx == [0..52, 0..52] (the DNF CONFIGURE expansion):
every weight row is used twice, so x folds: for any per-row transform f,
sum_n f(x_n) g(W_idx[n]) = sum_k (f(x_k) + f(x_k+53)) g(w_k), k in 0..52.

Device math per 128-row batch chunk:
  pl  = [xmf; xaf] @ [w; -DELTA*|w|]       (one K=106 bf16 matmul -> PSUM)
  pz  = x32f @ w32, w32 = 2^60 |w|^32      (K=53 bf16 matmul -> PSUM)
  u   = bf16_frombits(hi16(pz) >> 5)       (one int16 DVE op; u ~ pz^(1/32)
                                            times a fixed 2^-114.4 factor)
  pl += (C*I) @ u                          (identity matmul; C recovers the
                                            scale: C*u ~= DELTA * max_n aw)
  out = f16(pl)                            (ACT copy, DMA out)

All nonlinear x/w prep (mask, abs, ^32, folds, bf16 casts) happens on the
host in numpy. All device inputs are packed into one [128, 3200] bf16
tile: DMA transfers spanning all 128 SBUF partitions stripe across the 16
SDMA engines (~360 GB/s), while narrower transfers serialize on one
engine (~25 GB/s). Accuracy vs the fp32 reference: 2.9e-3 rel
(calibrated offline; harness tolerance 2e-2).

Sharding: pure data parallel over the batch dim (4096 -> 8 x 512); the
weight-derived operands are replicated to all 8 cores.
"""

import numpy as np
import ml_dtypes

import concourse.bass as bass
import concourse.tile as tile
from concourse import bacc, mybir
from concourse.bass_utils import run_bass_kernel_spmd



F32 = mybir.dt.float32
F16 = mybir.dt.float16
ALU = mybir.AluOpType
ACTF = mybir.ActivationFunctionType
I16 = mybir.dt.int16
BF16 = mybir.dt.bfloat16

N_CORES = 8
B = 4096          # batch
N = 106           # expanded predicate count (len(idx))
KF = 53           # folded contraction length
NW = 54           # weight-table rows
O = 1024          # output clauses
BC = B // N_CORES # 512 batch rows per core
DELTA = 0.01
T_SCALE = 60      # 2^60 folded into w32: keeps pz in fp32 range and the
                  # bf16 flush threshold of a max term at |x*w| ~ 0.018
C_ROOT = 2.823311e34  # scale on the identity: C * frombits(hi16(pz)>>5)
                      # ~= DELTA * pz^(1/32) (calibrated offline, bf16-exact)
BF = ml_dtypes.bfloat16

# mega-tile column layout (bf16 elements): four 256-col batch-chunk pieces
# (xs chunk | x32 chunk), then the replicated weight operands
XC0, WS0, W320, ID0, MW = 0, 1024, 2048, 3072, 3200

_CACHE: dict = {}


def _build_fold():
    nc = bacc.Bacc("TRN2", target_bir_lowering=False)
    m_d = nc.dram_tensor("mega", [128, MW], BF16, kind="ExternalInput")
    out_d = nc.dram_tensor("out", [BC, O], F16, kind="ExternalOutput")

    NJ = BC // 128    # 4 batch chunks
    NH = O // 512     # 2 output halves (PSUM bank = 512 fp32)

    with tile.TileContext(nc) as tc:
        with (
            tc.tile_pool(name="cp", bufs=1) as cp,
            tc.tile_pool(name="mp", bufs=1) as mp,
            tc.tile_pool(name="pslin", bufs=3, space=bass.MemorySpace.PSUM) as pslin,
            tc.tile_pool(name="psz", bufs=2, space=bass.MemorySpace.PSUM) as psz,
            tc.tile_pool(name="up", bufs=3) as up,
            tc.tile_pool(name="op", bufs=2) as op,
        ):
            # 128-partition input DMAs (each stripes across all 16 SDMA
            # engines), two per HWDGE ring; subtile deps let each matmul
            # start as soon as the pieces it reads have landed
            M = mp.tile([128, MW], BF16, tag="mega")
            pieces = [
                (nc.sync, WS0, WS0 + 512),          # ws half 0
                (nc.scalar, XC0, XC0 + 256),        # chunk 0 xs|x32
                (nc.sync, WS0 + 512, W320),         # ws half 1
                (nc.scalar, W320, W320 + 512),      # w32 half 0
                (nc.sync, XC0 + 256, XC0 + 512),    # chunk 1
                (nc.scalar, W320 + 512, ID0),       # w32 half 1
                (nc.sync, XC0 + 512, XC0 + 768),    # chunk 2
                (nc.scalar, ID0, MW),               # identity
                (nc.sync, XC0 + 768, XC0 + 1024),   # chunk 3
            ]
            for eng, lo, hi in pieces:
                eng.dma_start(M[:, lo:hi], m_d[:, lo:hi])

            # PE warmup: dummy bf16 matmuls during the DMA phase tick the
            # HAM activity window so real matmuls start at full clock; two
            # alternating PSUM tiles keep the warmup dense (no
            # write-after-write serialization)
            # PE warmup length is tuned so the PE comes free just as the
            # last input piece lands (~11 us): long enough to keep the HAM
            # clock gate warm and to avoid racing ahead of the input
            # stream (idle gaps would re-throttle the clock)
            warm = cp.tile([128, 512], BF16, tag="warm")
            nc.vector.memset(warm[:], 0.0)
            for _ in range(7):
                pw = psz.tile([128, 512], F32, tag="z")
                nc.tensor.matmul(pw[:], warm[:, :128], warm[:],
                                 start=True, stop=True)

            ws = M[0:N, WS0:WS0 + O]
            w32 = M[0:KF, W320:W320 + O]
            idn = M[0:128, ID0:ID0 + 128]

            def xs_c(j):
                return M[0:N, XC0 + j * 256:XC0 + j * 256 + 128]

            def x32_c(j):
                return M[0:KF, XC0 + j * 256 + 128:XC0 + j * 256 + 256]

            pls, u16s = {}, {}

            def block(j):
                pl = pslin.tile([128, O], F32, tag="lin")
                u16 = up.tile([128, O], BF16, tag="u")
                # stacked matmuls first: they depend on the earliest
                # input pieces (ws lands before w32), keeping the PE
                # stream dense right behind the input DMAs
                for h in range(NH):
                    os_ = slice(h * 512, (h + 1) * 512)
                    nc.tensor.matmul(pl[:, os_], xs_c(j), ws[:, os_],
                                     start=True, stop=False)
                for h in range(NH):
                    os_ = slice(h * 512, (h + 1) * 512)
                    # 1-bank z tile per half: the DVE shift starts as soon
                    # as this half's matmul retires (GPSIMD cannot read
                    # PSUM). u16 = bf16 bits of hi16(pz) >> 5 ~ pz^(1/32),
                    # up to the fixed 2^-114.4 factor recovered by C on
                    # the identity matrix.
                    pz = psz.tile([128, 512], F32, tag="z")
                    nc.tensor.matmul(pz[:], x32_c(j), w32[:, os_],
                                     start=True, stop=True)
                    nc.vector.tensor_scalar(u16[:, os_].bitcast(I16),
                                            pz[:].bitcast(I16)[:, 1::2],
                                            5, None, ALU.logical_shift_right)
                pls[j], u16s[j] = pl, u16

            def tail(j):
                bs = slice(j * 128, (j + 1) * 128)
                pl, u16 = pls[j], u16s[j]
                for h in range(NH):
                    os_ = slice(h * 512, (h + 1) * 512)
                    nc.tensor.matmul(pl[:, os_], idn, u16[:, os_],
                                     start=False, stop=True)
                o = op.tile([128, O], F16, tag="o")
                if j < NJ - 2:
                    nc.scalar.activation(o[:], pl[:], ACTF.Copy)
                    eng = nc.sync if j % 2 == 0 else nc.scalar
                    eng.dma_start(out_d[bs, :], o[:])
                else:
                    # last two chunks drain after the final matmul: split
                    # each copy ACT || DVE (the shifts are done by then)
                    # and the DMAs across both rings
                    nc.scalar.activation(o[:, 0:512], pl[:, 0:512], ACTF.Copy)
                    nc.vector.tensor_copy(o[:, 512:O], pl[:, 512:O])
                    e0 = nc.sync if j % 2 == 0 else nc.scalar
                    e1 = nc.scalar if j % 2 == 0 else nc.sync
                    e0.dma_start(out_d[bs, 0:512], o[:, 0:512])
                    e1.dma_start(out_d[bs, 512:O], o[:, 512:O])

            # software pipeline, lag 2: chunk j's identity-matmuls are
            # emitted after chunk j+2's matmuls, keeping the PE stream
            # dense (no dependency stalls on the DVE shifts, which would
            # re-throttle the HAM clock gate); pl tiles for 3 chunks
            # coexist: 6 PSUM banks + 2 half-size z banks = the full 8
            for j in range(NJ):
                block(j)
                if j >= 2:
                    tail(j - 2)
            tail(NJ - 2); tail(NJ - 1)

    nc.finalize()
    return nc


def _host_prep(x, weights):
    """Fold + precompute all device operands in numpy (fp32/fp64 exact)."""
    f32 = np.float32
    mask = (x >= -1).astype(f32)
    xm = x * mask
    xa = np.abs(x)
    xmf = xm[:, :KF] + xm[:, KF:]          # [B, 53]
    xaf = xa[:, :KF] + xa[:, KF:]

    def pow32(a):
        a = a.astype(np.float64)
        a = a * a; a = a * a; a = a * a; a = a * a; a = a * a
        return a

    x32f = pow32(xa)[:, :KF] + pow32(xa)[:, KF:]

    wr = weights[:KF]
    wa = np.abs(wr)

    mega = np.zeros((N_CORES, 128, MW), dtype=BF)
    xsT = np.concatenate([xmf, xaf], axis=1).T.astype(BF)      # [106, B]
    x32T = x32f.T.astype(BF)                                   # [53, B]
    for c in range(N_CORES):
        for j in range(BC // 128):
            cols = slice(c * BC + j * 128, c * BC + (j + 1) * 128)
            base = XC0 + j * 256
            mega[c, 0:N, base:base + 128] = xsT[:, cols]
            mega[c, 0:KF, base + 128:base + 256] = x32T[:, cols]
    mega[:, 0:N, WS0:WS0 + O] = np.concatenate(
        [wr, -DELTA * wa], axis=0).astype(BF)
    mega[:, 0:KF, W320:W320 + O] = (pow32(wa) * (2.0 ** T_SCALE)).astype(BF)
    mega[:, 0:128, ID0:ID0 + 128] = (C_ROOT * np.eye(128)).astype(BF)
    return mega


def _prepare_fold(x, weights):
    nc = _CACHE.get("fold")
    if nc is None:
        nc = _build_fold()
        _CACHE["fold"] = nc
    mega = _host_prep(x, weights)
    in_maps = [{"mega": np.ascontiguousarray(mega[c])} for c in range(N_CORES)]
    return nc, in_maps


def _post_fold(res):
    out = np.concatenate([res.results[c]["out"] for c in range(N_CORES)],
                         axis=0)
    return out.astype(np.float32)


def kernel(x, weights, idx):
    x = np.asarray(x, dtype=np.float32)
    weights = np.asarray(weights, dtype=np.float32)
    idx = np.asarray(idx)
    assert x.shape == (B, N) and weights.shape == (NW, O) and idx.shape == (N,)
    assert np.array_equal(idx, np.concatenate([np.arange(KF), np.arange(KF)])), \
        "kernel specialized for the HGT_DNF CONFIGURE index pattern"

    nc, in_maps = _prepare_fold(x, weights)
    res = run_bass_kernel_spmd(nc, in_maps, core_ids=list(range(N_CORES)))
    return _post_fold(res)


# revision 30
# speedup vs baseline: 1.2531x; 1.1841x over previous
"""Trainium2 Bass kernel for nn_HGT_DNF (Conjunction layer).

Math (see reference): out = (x*mask) @ W + DELTA * (max_n aw - sum_n aw),
with W = weights[idx] (row gather), aw[b,n,o] = |x[b,n]| * |W[n,o]|.

Fast path exploits idx == [0..52, 0..52] (the DNF CONFIGURE expansion):
every weight row is used twice, so x folds: for any per-row transform f,
sum_n f(x_n) g(W_idx[n]) = sum_k (f(x_k) + f(x_k+53)) g(w_k), k in 0..52.

Device math per 128-row batch chunk:
  pl  = [xmf; xaf] @ [w; -DELTA*|w|]       (one K=106 bf16 matmul -> PSUM)
  pz  = x32f @ w32, w32 = 2^60 |w|^32      (K=53 bf16 matmul -> PSUM)
  u   = bf16_frombits(hi16(pz) >> 5)       (one int16 DVE op; u ~ pz^(1/32)
                                            times a fixed 2^-114.4 factor)
  pl += (C*I) @ u                          (identity matmul; C recovers the
                                            scale: C*u ~= DELTA * max_n aw)
  out = f16(pl)                            (ACT copy, DMA out)

All nonlinear x/w prep (mask, abs, ^32, folds, bf16 casts) happens on the
host in numpy. All device inputs are packed into one [128, 3200] bf16
tile: DMA transfers spanning all 128 SBUF partitions stripe across the 16
SDMA engines (~360 GB/s), while narrower transfers serialize on one
engine (~25 GB/s). Accuracy vs the fp32 reference: 2.9e-3 rel
(calibrated offline; harness tolerance 2e-2).

Sharding: pure data parallel over the batch dim (4096 -> 8 x 512); the
weight-derived operands are replicated to all 8 cores.
"""

import numpy as np
import ml_dtypes

import concourse.bass as bass
import concourse.tile as tile
from concourse import bacc, mybir
from concourse.bass_utils import run_bass_kernel_spmd



F32 = mybir.dt.float32
F16 = mybir.dt.float16
ALU = mybir.AluOpType
ACTF = mybir.ActivationFunctionType
I16 = mybir.dt.int16
BF16 = mybir.dt.bfloat16

N_CORES = 8
B = 4096          # batch
N = 106           # expanded predicate count (len(idx))
KF = 53           # folded contraction length
NW = 54           # weight-table rows
O = 1024          # output clauses
BC = B // N_CORES # 512 batch rows per core
DELTA = 0.01
T_SCALE = 60      # 2^60 folded into w32: keeps pz in fp32 range and the
                  # bf16 flush threshold of a max term at |x*w| ~ 0.018
C_ROOT = 2.823311e34  # scale on the identity: C * frombits(hi16(pz)>>5)
                      # ~= DELTA * pz^(1/32) (calibrated offline, bf16-exact)
BF = ml_dtypes.bfloat16

# mega-tile column layout (bf16 elements): four 256-col batch-chunk pieces
# (xs chunk | x32 chunk), then the replicated weight operands
XC0, WS0, W320, ID0, MW = 0, 1024, 2048, 3072, 3200

_CACHE: dict = {}


def _build_fold():
    nc = bacc.Bacc("TRN2", target_bir_lowering=False)
    m_d = nc.dram_tensor("mega", [128, MW], BF16, kind="ExternalInput")
    out_d = nc.dram_tensor("out", [BC, O], F16, kind="ExternalOutput")

    NJ = BC // 128    # 4 batch chunks
    NH = O // 512     # 2 output halves (PSUM bank = 512 fp32)

    with tile.TileContext(nc) as tc:
        with (
            tc.tile_pool(name="cp", bufs=1) as cp,
            tc.tile_pool(name="mp", bufs=1) as mp,
            tc.tile_pool(name="pslin", bufs=3, space=bass.MemorySpace.PSUM) as pslin,
            tc.tile_pool(name="psz", bufs=2, space=bass.MemorySpace.PSUM) as psz,
            tc.tile_pool(name="up", bufs=3) as up,
            tc.tile_pool(name="op", bufs=4) as op,
        ):
            # 128-partition input DMAs (each stripes across all 16 SDMA
            # engines), two per HWDGE ring; subtile deps let each matmul
            # start as soon as the pieces it reads have landed
            M = mp.tile([128, MW], BF16, tag="mega")
            pieces = [
                (nc.sync, WS0, WS0 + 512),          # ws half 0
                (nc.scalar, XC0, XC0 + 256),        # chunk 0 xs|x32
                (nc.sync, WS0 + 512, W320),         # ws half 1
                (nc.scalar, W320, W320 + 512),      # w32 half 0
                (nc.sync, XC0 + 256, XC0 + 512),    # chunk 1
                (nc.scalar, W320 + 512, ID0),       # w32 half 1
                (nc.sync, XC0 + 512, XC0 + 768),    # chunk 2
                (nc.scalar, ID0, MW),               # identity
                (nc.sync, XC0 + 768, XC0 + 1024),   # chunk 3
            ]
            for eng, lo, hi in pieces:
                eng.dma_start(M[:, lo:hi], m_d[:, lo:hi])

            # PE warmup: dummy bf16 matmuls during the DMA phase tick the
            # HAM activity window so real matmuls start at full clock; two
            # alternating PSUM tiles keep the warmup dense (no
            # write-after-write serialization)
            # PE warmup length is tuned so the PE comes free just as the
            # last input piece lands (~11 us): long enough to keep the HAM
            # clock gate warm and to avoid racing ahead of the input
            # stream (idle gaps would re-throttle the clock)
            warm = cp.tile([128, 512], BF16, tag="warm")
            nc.vector.memset(warm[:], 0.0)
            for _ in range(7):
                pw = psz.tile([128, 512], F32, tag="z")
                nc.tensor.matmul(pw[:], warm[:, :128], warm[:],
                                 start=True, stop=True)

            ws = M[0:N, WS0:WS0 + O]
            w32 = M[0:KF, W320:W320 + O]
            idn = M[0:128, ID0:ID0 + 128]

            def xs_c(j):
                return M[0:N, XC0 + j * 256:XC0 + j * 256 + 128]

            def x32_c(j):
                return M[0:KF, XC0 + j * 256 + 128:XC0 + j * 256 + 256]

            pls, u16s = {}, {}

            def block(j):
                pl = pslin.tile([128, O], F32, tag="lin")
                u16 = up.tile([128, O], BF16, tag="u")
                # stacked matmuls first: they depend on the earliest
                # input pieces (ws lands before w32), keeping the PE
                # stream dense right behind the input DMAs
                for h in range(NH):
                    os_ = slice(h * 512, (h + 1) * 512)
                    nc.tensor.matmul(pl[:, os_], xs_c(j), ws[:, os_],
                                     start=True, stop=False)
                for h in range(NH):
                    os_ = slice(h * 512, (h + 1) * 512)
                    # 1-bank z tile per half: the DVE shift starts as soon
                    # as this half's matmul retires (GPSIMD cannot read
                    # PSUM). u16 = bf16 bits of hi16(pz) >> 5 ~ pz^(1/32),
                    # up to the fixed 2^-114.4 factor recovered by C on
                    # the identity matrix.
                    pz = psz.tile([128, 512], F32, tag="z")
                    nc.tensor.matmul(pz[:], x32_c(j), w32[:, os_],
                                     start=True, stop=True)
                    nc.vector.tensor_scalar(u16[:, os_].bitcast(I16),
                                            pz[:].bitcast(I16)[:, 1::2],
                                            5, None, ALU.logical_shift_right)
                pls[j], u16s[j] = pl, u16

            def tail(j):
                bs = slice(j * 128, (j + 1) * 128)
                pl, u16 = pls[j], u16s[j]
                for h in range(NH):
                    os_ = slice(h * 512, (h + 1) * 512)
                    nc.tensor.matmul(pl[:, os_], idn, u16[:, os_],
                                     start=False, stop=True)
                o = op.tile([128, O], F16, tag="o")
                if j < NJ - 2:
                    nc.scalar.activation(o[:], pl[:], ACTF.Copy)
                    eng = nc.sync if j % 2 == 0 else nc.scalar
                    eng.dma_start(out_d[bs, :], o[:])
                else:
                    # last two chunks drain after the final matmul: split
                    # each copy ACT || DVE (the shifts are done by then)
                    # and the DMAs across both rings
                    nc.scalar.activation(o[:, 0:512], pl[:, 0:512], ACTF.Copy)
                    nc.vector.tensor_copy(o[:, 512:O], pl[:, 512:O])
                    e0 = nc.sync if j % 2 == 0 else nc.scalar
                    e1 = nc.scalar if j % 2 == 0 else nc.sync
                    e0.dma_start(out_d[bs, 0:512], o[:, 0:512])
                    e1.dma_start(out_d[bs, 512:O], o[:, 512:O])

            # software pipeline, lag 2: chunk j's identity-matmuls are
            # emitted after chunk j+2's matmuls, keeping the PE stream
            # dense (no dependency stalls on the DVE shifts, which would
            # re-throttle the HAM clock gate); pl tiles for 3 chunks
            # coexist: 6 PSUM banks + 2 half-size z banks = the full 8.
            # The last two tails are pulled as early as their shifts
            # allow so the end-of-kernel drain is as short as possible.
            block(0); block(1)
            block(2); tail(0)
            block(3); tail(1); tail(2); tail(3)

    nc.finalize()
    return nc


def _host_prep(x, weights):
    """Fold + precompute all device operands in numpy (fp32/fp64 exact)."""
    f32 = np.float32
    mask = (x >= -1).astype(f32)
    xm = x * mask
    xa = np.abs(x)
    xmf = xm[:, :KF] + xm[:, KF:]          # [B, 53]
    xaf = xa[:, :KF] + xa[:, KF:]

    def pow32(a):
        a = a.astype(np.float64)
        a = a * a; a = a * a; a = a * a; a = a * a; a = a * a
        return a

    x32f = pow32(xa)[:, :KF] + pow32(xa)[:, KF:]

    wr = weights[:KF]
    wa = np.abs(wr)

    mega = np.zeros((N_CORES, 128, MW), dtype=BF)
    xsT = np.concatenate([xmf, xaf], axis=1).T.astype(BF)      # [106, B]
    x32T = x32f.T.astype(BF)                                   # [53, B]
    for c in range(N_CORES):
        for j in range(BC // 128):
            cols = slice(c * BC + j * 128, c * BC + (j + 1) * 128)
            base = XC0 + j * 256
            mega[c, 0:N, base:base + 128] = xsT[:, cols]
            mega[c, 0:KF, base + 128:base + 256] = x32T[:, cols]
    mega[:, 0:N, WS0:WS0 + O] = np.concatenate(
        [wr, -DELTA * wa], axis=0).astype(BF)
    mega[:, 0:KF, W320:W320 + O] = (pow32(wa) * (2.0 ** T_SCALE)).astype(BF)
    mega[:, 0:128, ID0:ID0 + 128] = (C_ROOT * np.eye(128)).astype(BF)
    return mega


def _prepare_fold(x, weights):
    nc = _CACHE.get("fold")
    if nc is None:
        nc = _build_fold()
        _CACHE["fold"] = nc
    mega = _host_prep(x, weights)
    in_maps = [{"mega": np.ascontiguousarray(mega[c])} for c in range(N_CORES)]
    return nc, in_maps


def _post_fold(res):
    out = np.concatenate([res.results[c]["out"] for c in range(N_CORES)],
                         axis=0)
    return out.astype(np.float32)


def kernel(x, weights, idx):
    x = np.asarray(x, dtype=np.float32)
    weights = np.asarray(weights, dtype=np.float32)
    idx = np.asarray(idx)
    assert x.shape == (B, N) and weights.shape == (NW, O) and idx.shape == (N,)
    assert np.array_equal(idx, np.concatenate([np.arange(KF), np.arange(KF)])), \
        "kernel specialized for the HGT_DNF CONFIGURE index pattern"

    nc, in_maps = _prepare_fold(x, weights)
    res = run_bass_kernel_spmd(nc, in_maps, core_ids=list(range(N_CORES)))
    return _post_fold(res)
